# revision 13
# baseline (speedup 1.0000x reference)
"""Trainium2 Bass kernel for nn_DynamicHead (varying-coefficient spline MLP), v2.

Math: basis(t) = [1,t,t^2,t^3, relu(t-k_j)^3 ...] (12 fns, 8 knots at j/9).
Each vc_layer: out = sum_s basis_s * (x @ W_s) + basis @ b.
Within knot segment m, basis collapses to powers [1,t,t^2,t^3] against
segment-combined weights C[m,p] = sum_s gamma[m,p,s] W_s; each layer becomes a
K=4*256 matmul over z[(p,i), b] = t^p * x[i, b], samples grouped by segment.

v2 changes vs v1: fp16 operands everywhere on the PE (same 1 cyc/row as fp32r
but half the DMA + 2x DVE mode for z-builds), per-segment capacities (cuts the
~12% uniform-cap padding), weights on the sync HWDGE queue with explicit
one-step prefetch, activations/broadcast rows on the scalar HWDGE queue
(SBUF-to-SBUF broadcast of t-power rows; no GpSimd SWDGE anywhere), and
c2/ones loaded once for all segments.

Host: sort samples by segment, deal round-robin across 8 cores (counts per
core differ by <=1), per-segment cap = padded max count; prepack weights into
SBUF tile layouts in fp16. Device (per core, SPMD): segment-skewed software
pipeline L0(s) | L1(s-1) | head(s-2).
"""
import os
import sys
import types

for _p in ('/opt/trn_rl_repo', '/root/.axon_site/_ro/trn_rl_repo'):
    if _p not in sys.path:
        sys.path.append(_p)

import numpy as np
import concourse.bass as bass
import concourse.tile as tile
from concourse import bacc, mybir
from concourse import bass_utils

F32 = mybir.dt.float32
F16 = mybir.dt.float16
RELU = mybir.ActivationFunctionType.Relu
COPY = mybir.ActivationFunctionType.Copy
IDENT = mybir.ActivationFunctionType.Identity

B, D, NSEG, NSB = 32768, 256, 9, 4
N_CORES = 8
KNOTS = np.array([i / 9.0 for i in range(1, 9)], dtype=np.float64)
SDIM = 12
NKT = NSB * D // 128                   # 8 k-tiles of 128

# set True by test harness for a profiled run
TRACE = False
LAST_EXEC_NS = None
LAST_MEAN_EXEC_NS = None
LAST_RES = None

_PROG_CACHE = {}


def _register_ntff_hook():
    try:
        import antenv.axon_hooks  # noqa: F401
        return
    except ImportError:
        pass
    try:
        from trn_agent_boot.trn_boot import _ntff_profile_via_ctypes
        hook = _ntff_profile_via_ctypes('/opt/axon/libaxon_pjrt.so')
        mod = types.ModuleType('antenv.axon_hooks')
        mod.get_axon_ntff_profile_hook = lambda: hook
        sys.modules['antenv.axon_hooks'] = mod
    except Exception:
        pass


def _gamma() -> np.ndarray:
    """(NSEG, NSB, SDIM): basis -> per-segment cubic coefficients."""
    g = np.zeros((NSEG, NSB, SDIM), dtype=np.float64)
    for m in range(NSEG):
        for p in range(NSB):
            g[m, p, p] = 1.0
        for j in range(1, 9):          # spline s = 3 + j, knot k = j/9
            if j <= m:
                k = KNOTS[j - 1]
                g[m, 0, 3 + j] = -k ** 3
                g[m, 1, 3 + j] = 3 * k ** 2
                g[m, 2, 3 + j] = -3 * k
                g[m, 3, 3 + j] = 1.0
    return g


def _build_program(caps):
    """Compile the SPMD single-core program for per-segment capacities caps."""
    caps = list(caps)
    cmax = max(caps)
    offs = [0]
    for c in caps:
        offs.append(offs[-1] + c)
    bp = offs[-1]

    nc = bacc.Bacc("TRN2", target_bir_lowering=False, debug=False,
                   num_devices=N_CORES)

    # cw: per (layer, seg) prepacked (128, 2304): 8 k-tile blocks of 256 (o)
    # cols + bias block (partitions 0..3) at cols 2048..2303.
    xT_ap = nc.dram_tensor("xT", [D, bp], F16, kind="ExternalInput").ap()
    tp_ap = nc.dram_tensor("tp", [NSB, bp], F16, kind="ExternalInput").ap()
    c0w_ap = nc.dram_tensor("c0w", [NSEG, 128, (NKT + 1) * D], F16, kind="ExternalInput").ap()
    c1w_ap = nc.dram_tensor("c1w", [NSEG, 128, (NKT + 1) * D], F16, kind="ExternalInput").ap()
    # c2w: (128, NSEG*2*NSB): per seg 8 cols = 2 h-blocks of 4; c2b: (NSB, NSEG)
    c2w_ap = nc.dram_tensor("c2w", [128, NSEG * 2 * NSB], F16, kind="ExternalInput").ap()
    c2b_ap = nc.dram_tensor("c2b", [NSB, NSEG], F32, kind="ExternalInput").ap()
    ones_ap = nc.dram_tensor("ones4", [NSB, 1], F16, kind="ExternalInput").ap()
    out_ap = nc.dram_tensor("out", [1, bp], F32, kind="ExternalOutput").ap()

    cw_ap = (c0w_ap, c1w_ap)

    WCOLS = (NKT + 1) * D
    # weight chunk groups: seg 0 alone (fast pipeline start), then the rest
    WCH = [(0, 1), (1, 3), (3, 6), (6, 9)]
    SCH = [(0, 1), (1, 9)]               # xin / bcast / tp chunk groups

    with tile.TileContext(nc) as tc:
        with (
            tc.tile_pool(name="act", bufs=1) as actp,
            tc.tile_pool(name="bc", bufs=1) as bcp,
            tc.tile_pool(name="z", bufs=1) as zp,
            tc.tile_pool(name="w", bufs=1) as wp,
            tc.tile_pool(name="sm", bufs=1) as smp,
            tc.tile_pool(name="pm", bufs=1, space="PSUM") as pmp,
            tc.tile_pool(name="pq", bufs=1, space="PSUM") as pqp,
        ):
            # ---- whole-problem resident tiles, loaded via a few big DMAs ----
            wtile = {}                    # (L, chunk) -> tile over segs lo..hi
            wbase = {}                    # (L, s) -> (tile, col base)
            xint = {}                     # chunk -> tile ; xbase: s -> (tile, col)
            xbase, bcbase, tpbase = {}, {}, {}

            def issue_w(L, ci):
                lo, hi = WCH[ci]
                tl = wp.tile([128, (hi - lo) * WCOLS], F16,
                             name=f"w{L}c{ci}", tag=f"w{L}c{ci}")
                eng = nc.sync if L == 0 else nc.scalar
                eng.dma_start(
                    tl[:, :].rearrange("p (s c) -> p s c", s=hi - lo),
                    cw_ap[L][lo:hi].transpose([1, 0, 2]))
                for s in range(lo, hi):
                    wbase[(L, s)] = (tl, (s - lo) * WCOLS)

            def issue_xin(ci):
                lo, hi = SCH[ci]
                w = 2 * (offs[hi] - offs[lo])
                tl = actp.tile([128, w], F16, name=f"xin{ci}", tag=f"xin{ci}")
                nc.sync.dma_start(
                    tl[:, :].rearrange("p (h b) -> p h b", h=2),
                    xT_ap[:, offs[lo]:offs[hi]].rearrange("(h p) b -> p h b", p=128))
                for s in range(lo, hi):
                    xbase[s] = (tl, offs[s] - offs[lo], offs[hi] - offs[lo])

            def issue_bc(ci):
                lo, hi = SCH[ci]
                w = offs[hi] - offs[lo]
                tl = bcp.tile([128, (NSB - 1) * w], F16, name=f"bc{ci}",
                              tag=f"bc{ci}")
                nc.gpsimd.dma_start(
                    tl[:, :].rearrange("q (p b) -> q p b", p=NSB - 1),
                    tp_ap[1:NSB, offs[lo]:offs[hi]].partition_broadcast(128))
                for s in range(lo, hi):
                    bcbase[s] = (tl, offs[s] - offs[lo], w)

            def issue_tp(ci):
                lo, hi = SCH[ci]
                w = offs[hi] - offs[lo]
                tl = smp.tile([NSB, w], F16, name=f"tp{ci}", tag=f"tp{ci}")
                nc.scalar.dma_start(tl[:, :], tp_ap[:, offs[lo]:offs[hi]])
                for s in range(lo, hi):
                    tpbase[s] = (tl, offs[s] - offs[lo])

            def tp_sl(s):
                tl, o = tpbase[s]
                return tl[:, o:o + caps[s]]

            x1, x2 = {}, {}

            def vc_layer(s, L, store):
                """layers 0/1: (o,b) = relu(C.T @ z + Cb.T @ tp), feature-major"""
                c = caps[s]
                w, wb = wbase.pop((L, s))
                if L == 0:
                    xt, so, xw = xbase[s]
                    xin_t = xt[:, :].rearrange(
                        "p (h b) -> p h b", h=2)[:, :, so:so + c]
                else:
                    xin_t = x1[s][:, :2 * c].rearrange("p (h b) -> p h b", h=2)
                bt, bo, bw = bcbase[s]
                zt = zp.tile([128, (NSB - 1) * 2 * cmax], F16,
                             name=f"z{L}_{s}", tag="z", bufs=3)
                for p in range(1, NSB):
                    blk = (p - 1) * 2 * c
                    nc.vector.tensor_mul(
                        zt[:, blk:blk + 2 * c].rearrange("q (h b) -> q h b", h=2),
                        xin_t[:, :, :c],
                        bt[:, (p - 1) * bw + bo:(p - 1) * bw + bo + c]
                        .unsqueeze(1).broadcast_to([128, 2, c]))

                # p=0 k-tiles (xin only) for both m first, so the PE can run
                # while the z-build is still in flight on DVE
                pst = []
                for m in range(2):
                    ps = pmp.tile([128, cmax], F32, name=f"pm{L}_{s}_{m}",
                                  tag="pm", bufs=5)
                    for kt in range(2):
                        nc.tensor.matmul(
                            ps[:, :c],
                            w[:, wb + kt * D + m * 128:wb + kt * D + (m + 1) * 128],
                            xin_t[:, kt, :c],
                            start=(kt == 0), stop=False)
                    pst.append(ps)
                outs = []
                for m in range(2):
                    ps = pst[m]
                    for kt in range(2, NKT):
                        p, h = divmod(kt, 2)
                        blk = ((p - 1) * 2 + h) * c
                        nc.tensor.matmul(
                            ps[:, :c],
                            w[:, wb + kt * D + m * 128:wb + kt * D + (m + 1) * 128],
                            zt[:, blk:blk + c], start=False, stop=False)
                    nc.tensor.matmul(
                        ps[:, :c],
                        w[0:NSB, wb + NKT * D + m * 128:wb + NKT * D + (m + 1) * 128],
                        tp_sl(s), start=False, stop=True)
                    outs.append((m, ps))
                xo = actp.tile([128, 2 * cmax], F16, name=f"x{L + 1}_{s}",
                               tag=f"xo{L}", bufs=3)
                for m, ps in outs:
                    nc.scalar.activation(xo[:, m * c:(m + 1) * c],
                                         ps[:, :c], RELU)
                store[s] = xo

            def head_layer(s):
                """layer 2 (out_dim=1): q=C2.T@x2 (+b2), out = ones.T @ (q*tp)"""
                c = caps[s]
                psq = pqp.tile([NSB, cmax], F32, name=f"pq{s}", tag="pq", bufs=2)
                for h in range(2):
                    nc.tensor.matmul(psq[:, :c],
                                     c2wt[:, s * 2 * NSB + h * NSB:s * 2 * NSB + (h + 1) * NSB],
                                     x2[s][:, h * c:(h + 1) * c],
                                     start=(h == 0), stop=(h == 1))
                qb = smp.tile([NSB, cmax], F16, name=f"qb{s}", tag="qb", bufs=3)
                nc.scalar.activation(qb[:, :c], psq[:, :c], IDENT,
                                     bias=c2bt[:, s:s + 1])
                rq = smp.tile([NSB, cmax], F16, name=f"rq{s}", tag="rq", bufs=3)
                nc.vector.tensor_mul(rq[:, :c], qb[:, :c], tp_sl(s))
                psr = pqp.tile([1, cmax], F32, name=f"pr{s}", tag="pr", bufs=1)
                nc.tensor.matmul(psr[:, :c], ones4[:, :], rq[:, :c],
                                 start=True, stop=True)
                orow = smp.tile([1, cmax], F32, name=f"or{s}", tag="or", bufs=3)
                nc.scalar.activation(orow[:, :c], psr[:, :c], COPY)
                nc.gpsimd.dma_start(out_ap[0:1, offs[s]:offs[s] + c],
                                    orow[:, :c])

            # ---- prologue: all DMAs issued up front, seg-0 chunks first ----
            issue_w(0, 0)                 # sync
            issue_xin(0)                  # sync
            issue_bc(0)                   # gpsimd
            issue_w(1, 0)                 # scalar
            issue_tp(0)                   # scalar
            issue_w(0, 1)
            issue_w(1, 1)
            issue_xin(1)
            issue_bc(1)
            issue_tp(1)
            issue_w(0, 2)
            issue_w(1, 2)
            ones4 = smp.tile([NSB, 1], F16, name="ones4", tag="ones4")
            nc.scalar.dma_start(ones4[:, :], ones_ap[:, :])
            c2wt = smp.tile([128, NSEG * 2 * NSB], F16, name="c2w", tag="c2w")
            nc.scalar.dma_start(c2wt[:, :], c2w_ap[:, :])
            c2bt = smp.tile([NSB, NSEG], F32, name="c2b", tag="c2b")
            nc.scalar.dma_start(c2bt[:, :], c2b_ap[:, :])
            issue_w(0, 3)
            issue_w(1, 3)

            # segment-skewed software pipeline: L0(s+1) overlaps L1(s)/L2(s-1)
            for step in range(NSEG + 2):
                if step < NSEG:
                    vc_layer(step, 0, x1)
                if 1 <= step < NSEG + 1:
                    vc_layer(step - 1, 1, x2)
                    x1.pop(step - 1)
                if step >= 2:
                    head_layer(step - 2)
                    x2.pop(step - 2)

    nc.compile()
    return nc


def _prep_host(treatment, features, W0, b0, W1, b1, W2, b2):
    t = np.asarray(treatment, dtype=np.float32)
    x = np.asarray(features, dtype=np.float32)
    seg = np.searchsorted(KNOTS.astype(np.float32), t, side='right')

    # deal each segment round-robin across cores
    core_of = np.empty(B, dtype=np.int64)
    pos_of = np.empty(B, dtype=np.int64)
    counts = np.zeros((N_CORES, NSEG), dtype=np.int64)
    for m in range(NSEG):
        idx = np.nonzero(seg == m)[0]
        for c in range(N_CORES):
            sub = idx[c::N_CORES]
            core_of[sub] = c
            pos_of[sub] = np.arange(len(sub))
            counts[c, m] = len(sub)
    caps = tuple(max(256, int(-(-counts[:, m].max() // 16) * 16))
                 for m in range(NSEG))
    offs = np.concatenate([[0], np.cumsum(caps)])
    bp = int(offs[-1])

    gather = np.full((N_CORES, bp), -1, dtype=np.int64)
    slot = offs[seg] + pos_of
    gather[core_of, slot] = np.arange(B)

    xT = np.zeros((N_CORES, D, bp), dtype=np.float16)
    tp = np.zeros((N_CORES, NSB, bp), dtype=np.float16)
    for c in range(N_CORES):
        v = gather[c] >= 0
        gi = gather[c][v]
        xT[c][:, v] = x[gi].T.astype(np.float16)
        tv = t[gi].astype(np.float64)
        tp[c][:, v] = np.stack([tv ** p for p in range(NSB)]).astype(np.float16)

    g = _gamma()
    cw = []
    for W, b in ((W0, b0), (W1, b1)):
        Ws = np.asarray(W, dtype=np.float64).reshape(SDIM, D, D)
        cc = np.einsum('mps,sio->mpio', g, Ws).reshape(NSEG, NSB * D, D)
        cb = np.einsum('mps,so->mpo', g, np.asarray(b, np.float64))
        packed = np.zeros((NSEG, 128, (NKT + 1) * D), dtype=np.float16)
        for kt in range(NKT):
            packed[:, :, kt * D:(kt + 1) * D] = cc[:, kt * 128:(kt + 1) * 128, :]
        packed[:, 0:NSB, NKT * D:] = cb
        cw.append(packed)
    c2w = np.einsum('mps,si->mip', g, np.asarray(W2, np.float64))   # (9, 256, 4)
    c2b = np.einsum('mps,s->mp', g, np.asarray(b2, np.float64)[:, 0])  # (9,4)
    c2wp = np.zeros((128, NSEG * 2 * NSB), dtype=np.float16)
    for s in range(NSEG):
        for h in range(2):
            c2wp[:, s * 2 * NSB + h * NSB:s * 2 * NSB + (h + 1) * NSB] = \
                c2w[s, h * 128:(h + 1) * 128, :]

    shared = dict(c0w=np.ascontiguousarray(cw[0]), c1w=np.ascontiguousarray(cw[1]),
                  c2w=c2wp, c2b=np.ascontiguousarray(c2b.T.astype(np.float32)),
                  ones4=np.ones((NSB, 1), np.float16))
    in_maps = [dict(shared, xT=np.ascontiguousarray(xT[c]),
                    tp=np.ascontiguousarray(tp[c])) for c in range(N_CORES)]
    return caps, in_maps, gather


def kernel(treatment, features, W0, b0, W1, b1, W2, b2):
    global LAST_EXEC_NS, LAST_MEAN_EXEC_NS, LAST_RES
    caps, in_maps, gather = _prep_host(treatment, features, W0, b0, W1, b1, W2, b2)

    if caps not in _PROG_CACHE:
        _PROG_CACHE[caps] = _build_program(caps)
    nc = _PROG_CACHE[caps]

    if TRACE:
        _register_ntff_hook()
    res = bass_utils.run_bass_kernel_spmd(
        nc, in_maps, core_ids=list(range(N_CORES)), trace=TRACE)
    LAST_EXEC_NS = res.exec_time_ns
    LAST_MEAN_EXEC_NS = res.mean_exec_time_ns
    LAST_RES = res

    out = np.empty((B,), dtype=np.float32)
    for c in range(N_CORES):
        row = res.results[c]["out"][0]
        v = gather[c] >= 0
        out[gather[c][v]] = row[v]
    return out.reshape(B, 1)
